# revision 14
# baseline (speedup 1.0000x reference)
"""F2NetHead Trainium2 kernel (8 NeuronCores, Bass/Tile).

Reference computation (per batch b):
    qog = x @ W_qog.T + b_qog ; Q,O,G = split(qog)
    cq  = silu(conv1d(Q, conv_w, pad=1) + conv_b)          # mixes channels
    l   = (cq @ w_a.T) / sqrt(d)
    attn= softmax(l, axis=seq)
    glob= sum_seq(Q * attn)                                 # [1, d]
    P   = O * glob
    L   = silu(G) * cumsum(P, axis=seq)
    R   = L @ W_out.T + b_out

Sharding: 8 cores = 4 batches x 2 sequence halves. Each core computes
2048 tokens of one batch. The host supplies the x-rows with a 1-token
halo on each side (zero rows at the sequence edges) so the conv needs no
neighbor exchange. The only cross-core communication is a pairwise
AllReduce of 3 small [d] vectors per batch:
    E  = sum_seq exp(l)            (softmax denominator)
    N  = sum_seq Q * exp(l)        (softmax numerator of glob)
    sx = sum of this half's x rows (first half only)
The cumsum offset of the second half is glob * (W_O @ sx_h0 + T*b_O),
i.e. the first half's P-column-sums, computed without materializing O.

On-chip layout is feature-major ([d partitions, tokens free]) so every
sequence-axis op (softmax sums, global sum, cumsum) is a free-dim op.
All matmuls run in float32r (full PE rate, ~1.6e-4 rel err on hw).
"""

import numpy as np

import concourse.bacc as bacc
import concourse.mybir as mybir
import concourse.tile as tile
from concourse.bass_utils import run_bass_kernel_spmd

F32 = mybir.dt.float32
F32R = mybir.dt.float32r
AF = mybir.ActivationFunctionType
OP = mybir.AluOpType

B, S, D, DM = 4, 4096, 1024, 1024
N_CORES = 8
T = S // 2            # tokens per core
TH = T + 2            # with halo
DT = D // 128         # d tiles (8)
KT = DM // 128        # contraction tiles (8)
ABLK = 410            # phase A token block (5 blocks over TH=2050)
BBLK = 512            # phase B token block (4 blocks over T)
CBLK = 256            # phase C token block (8 blocks over T)
SCALE = 1.0 / float(np.sqrt(D))


def _emit(tc, nc, prm, phases=5):
    x, wqt, wct, wat, wot = prm["x"], prm["wqt"], prm["wct"], prm["wat"], prm["wot"]
    bq, bo, bg, cb, bout = prm["bq"], prm["bo"], prm["bg"], prm["cb"], prm["bout"]
    hf0, hf1, r_out = prm["hf0"], prm["hf1"], prm["r"]

    with (
        tc.tile_pool(name="cols", bufs=1) as cols,
        tc.tile_pool(name="dram", bufs=1, space="DRAM") as dram,
    ):
        # per-partition bias / flag columns ([128, DT] with d = a*128 + p)
        bq_sb = cols.tile([128, DT], F32)
        bo_sb = cols.tile([128, DT], F32)
        bg_sb = cols.tile([128, DT], F32)
        cb_sb = cols.tile([128, DT], F32)
        bout_sb = cols.tile([128, DT], F32)
        hf0_sb = cols.tile([128, 1], F32)
        hf1_sb = cols.tile([128, 1], F32)
        for t_, d_ in ((bq_sb, bq), (bo_sb, bo), (bg_sb, bg), (cb_sb, cb),
                       (bout_sb, bout), (hf0_sb, hf0), (hf1_sb, hf1)):
            nc.sync.dma_start(t_[:], d_[:])

        # accumulators that survive across phases
        sx_cols = cols.tile([128, KT, 5], F32)      # per-A-block x sums
        e_cols = cols.tile([128, DT * 4], F32)      # per-(a,B-block) exp sums
        n_cols = cols.tile([128, DT * 4], F32)      # per-(a,B-block) Q*exp sums
        stage = cols.tile([128, 3 * DT], F32)       # allreduce staging
        red = cols.tile([128, 3 * DT], F32)         # allreduce result
        glob = cols.tile([128, DT], F32)
        offset = cols.tile([128, DT], F32)

        # ---------------- phase A: Q^T over TH halo'd tokens ----------------
        with tc.tile_pool(name="qt", bufs=1) as qt_pool:
            qt = qt_pool.tile([128, DT, TH], F32R)
            with (
                tc.tile_pool(name="wq", bufs=1) as wq_pool,
                tc.tile_pool(name="xa", bufs=2) as xa_pool,
                tc.tile_pool(name="psa", bufs=8, space="PSUM") as psa,
            ):
                wq = wq_pool.tile([128, KT, DT, 128], F32R)
                for kc in range(KT):
                    nc.sync.dma_start(
                        wq[:, kc, :, :],
                        wqt[kc * 128:(kc + 1) * 128, 0:D]
                        .rearrange("p (a m) -> p a m", m=128).bitcast(F32R),
                    )
                for blk in range(5):
                    t0 = blk * ABLK
                    xt = xa_pool.tile([128, KT, ABLK], F32R, tag="xa")
                    for kc in range(KT):
                        nc.sync.dma_start(
                            xt[:, kc, :],
                            x[t0:t0 + ABLK, kc * 128:(kc + 1) * 128]
                            .rearrange("t p -> p t").bitcast(F32R),
                        )
                    # x column-sums over main (non-halo) tokens for cumsum offset
                    lo = 1 - t0 if t0 < 1 else 0
                    hi = ABLK - max(0, t0 + ABLK - (TH - 1))
                    nc.vector.tensor_reduce(
                        sx_cols[:, :, blk:blk + 1], xt[:, :, lo:hi],
                        axis=mybir.AxisListType.X, op=OP.add,
                    )
                    for a in range(DT):
                        ps = psa.tile([128, ABLK], F32, tag="ps")
                        for kc in range(KT):
                            nc.tensor.matmul(
                                ps[:], wq[:, kc, a, :], xt[:, kc, :],
                                start=(kc == 0), stop=(kc == KT - 1),
                            )
                        nc.vector.tensor_scalar_add(
                            qt[:, a, t0:t0 + ABLK], ps[:], bq_sb[:, a:a + 1]
                        )

            if phases == 1:
                for a in range(DT):
                    nc.sync.dma_start(
                        r_out[0:T, a * 128:(a + 1) * 128].rearrange("t p -> p t"),
                        qt[:, a, 1:T + 1].bitcast(F32),
                    )
                return

            # ------------- phase B1: cq^T = silu(conv(Q)) -------------
            with tc.tile_pool(name="cq", bufs=1) as cq_pool:
                cq = cq_pool.tile([128, DT, T], F32R)
                with (
                    tc.tile_pool(name="wc", bufs=2) as wc_pool,
                    tc.tile_pool(name="psb", bufs=8, space="PSUM") as psb,
                ):
                    for a in range(DT):
                        wc = wc_pool.tile([128, 3, KT, 128], F32R, tag="wc")
                        for k3 in range(3):
                            nc.sync.dma_start(
                                wc[:, k3, :, :],
                                wct[k3, :, a * 128:(a + 1) * 128]
                                .rearrange("(kc p) m -> p kc m", p=128)
                                .bitcast(F32R),
                            )
                        for blk in range(T // BBLK):
                            t0 = blk * BBLK
                            ps = psb.tile([128, BBLK], F32, tag="ps")
                            first = True
                            for k3 in range(3):
                                for kc in range(KT):
                                    nc.tensor.matmul(
                                        ps[:], wc[:, k3, kc, :],
                                        qt[:, kc, t0 + k3:t0 + k3 + BBLK],
                                        start=first,
                                        stop=(k3 == 2 and kc == KT - 1),
                                    )
                                    first = False
                            sig = wc_pool.tile([128, BBLK], F32, tag="sig")
                            nc.scalar.activation(
                                sig[:], ps[:], AF.Sigmoid, bias=cb_sb[:, a:a + 1]
                            )
                            nc.vector.scalar_tensor_tensor(
                                cq[:, a, t0:t0 + BBLK], ps[:], cb_sb[:, a:a + 1],
                                sig[:], OP.add, OP.mult,
                            )

                if phases == 2:
                    for a in range(DT):
                        nc.sync.dma_start(
                            r_out[0:T, a * 128:(a + 1) * 128]
                            .rearrange("t p -> p t"),
                            cq[:, a, :].bitcast(F32),
                        )
                    return

                # ------- phase B2: E/N partial sums from exp(logits) -------
                with (
                    tc.tile_pool(name="wa", bufs=1) as wa_pool,
                    tc.tile_pool(name="ex", bufs=4) as ex_pool,
                    tc.tile_pool(name="psl", bufs=8, space="PSUM") as psl,
                ):
                    wa = wa_pool.tile([128, KT, DT, 128], F32R)
                    for kc in range(KT):
                        nc.sync.dma_start(
                            wa[:, kc, :, :],
                            wat[kc * 128:(kc + 1) * 128, :]
                            .rearrange("p (a m) -> p a m", m=128).bitcast(F32R),
                        )
                    for blk in range(T // BBLK):
                        t0 = blk * BBLK
                        for a in range(DT):
                            ps = psl.tile([128, BBLK], F32, tag="ps")
                            for kc in range(KT):
                                nc.tensor.matmul(
                                    ps[:], wa[:, kc, a, :], cq[:, kc, t0:t0 + BBLK],
                                    start=(kc == 0), stop=(kc == KT - 1),
                                )
                            expl = ex_pool.tile([128, BBLK], F32, tag="expl")
                            idx = a * 4 + blk
                            nc.scalar.activation(
                                expl[:], ps[:], AF.Exp, scale=SCALE,
                                accum_out=e_cols[:, idx:idx + 1],
                            )
                            prod = ex_pool.tile([128, BBLK], F32, tag="prod")
                            nc.vector.scalar_tensor_tensor(
                                prod[:], expl[:], 0.0,
                                qt[:, a, t0 + 1:t0 + 1 + BBLK].bitcast(F32),
                                OP.add, OP.mult,
                                accum_out=n_cols[:, idx:idx + 1],
                            )

        if phases == 3:
            nc.sync.dma_start(
                r_out[0:32, 0:128].rearrange("t p -> p t"), e_cols[:]
            )
            nc.sync.dma_start(
                r_out[32:64, 0:128].rearrange("t p -> p t"), n_cols[:]
            )
            return

        # ---------------- allreduce E, N, sx over the seq pair ----------------
        nc.vector.tensor_reduce(
            stage[:, 0:DT], e_cols[:].rearrange("p (a b) -> p a b", b=4),
            axis=mybir.AxisListType.X, op=OP.add,
        )
        nc.vector.tensor_reduce(
            stage[:, DT:2 * DT], n_cols[:].rearrange("p (a b) -> p a b", b=4),
            axis=mybir.AxisListType.X, op=OP.add,
        )
        # x sums (main tokens) masked to the first half: slot = sx * (1-h)
        nc.vector.tensor_reduce(
            stage[:, 2 * DT:3 * DT], sx_cols[:],
            axis=mybir.AxisListType.X, op=OP.add,
        )
        nc.vector.tensor_scalar_mul(
            stage[:, 2 * DT:3 * DT], stage[:, 2 * DT:3 * DT], hf0_sb[:, 0:1]
        )
        cc_in = dram.tile([128, 3 * DT], F32)
        cc_out = dram.tile([128, 3 * DT], F32)
        nc.sync.dma_start(cc_in[:], stage[:])
        nc.gpsimd.collective_compute(
            "AllReduce", OP.add,
            replica_groups=[[0, 1], [2, 3], [4, 5], [6, 7]],
            ins=[cc_in.opt()], outs=[cc_out.opt()],
        )
        nc.sync.dma_start(red[:], cc_out[:])

        # glob = N / E
        recip = cols.tile([128, DT], F32)
        nc.vector.reciprocal(recip[:], red[:, 0:DT])
        nc.vector.tensor_mul(glob[:], red[:, DT:2 * DT], recip[:])

        # ---------------- phase C: O,G -> P -> cumsum -> L -> R ----------------
        with (
            tc.tile_pool(name="wog", bufs=1) as wog_pool,
            tc.tile_pool(name="wo2", bufs=1) as wo2_pool,
        ):
            wog = wog_pool.tile([128, KT, 2 * DT, 128], F32R)
            for kc in range(KT):
                nc.sync.dma_start(
                    wog[:, kc, :, :],
                    wqt[kc * 128:(kc + 1) * 128, D:3 * D]
                    .rearrange("p (a m) -> p a m", m=128).bitcast(F32R),
                )
            wo2 = wo2_pool.tile([128, KT, DT, 128], F32R)
            for kc in range(KT):
                nc.sync.dma_start(
                    wo2[:, kc, :, :],
                    wot[kc * 128:(kc + 1) * 128, :]
                    .rearrange("p (a m) -> p a m", m=128).bitcast(F32R),
                )

            # cumsum offset for the second half: glob * (W_O @ sx_h0 + T*b_O)
            # (plain fp32 matmul: fp32r rejects a size-1 moving operand)
            bo_t = cols.tile([128, DT], F32)
            nc.vector.tensor_scalar_mul(bo_t[:], bo_sb[:], float(T))
            offv = cols.tile([128, DT], F32)
            with tc.tile_pool(name="psm", bufs=2, space="PSUM") as psm:
                for a in range(DT):
                    ps = psm.tile([128, 1], F32, tag="ps")
                    for kc in range(KT):
                        nc.tensor.matmul(
                            ps[:], wog[:, kc, a, :].bitcast(F32),
                            red[:, 2 * DT + kc:2 * DT + kc + 1],
                            start=(kc == 0), stop=(kc == KT - 1),
                        )
                    nc.vector.tensor_scalar_add(
                        offv[:, a:a + 1], ps[:], bo_t[:, a:a + 1]
                    )
            nc.vector.tensor_mul(offset[:], offv[:], glob[:])
            nc.vector.tensor_scalar_mul(offset[:], offset[:], hf1_sb[:, 0:1])

            if phases == 4:
                nc.sync.dma_start(
                    r_out[0:DT, 0:128].rearrange("t p -> p t"), offset[:]
                )
                nc.sync.dma_start(
                    r_out[DT:2 * DT, 0:128].rearrange("t p -> p t"), glob[:]
                )
                return

            with (
                tc.tile_pool(name="xc", bufs=2) as xc_pool,
                tc.tile_pool(name="blkb", bufs=2) as blk_pool,
                tc.tile_pool(name="psc", bufs=8, space="PSUM") as psc,
            ):
                c_prev = None
                nblk = T // CBLK if phases >= 5 else 0
                if phases >= 50:
                    nblk = phases - 50
                for blk in range(nblk):
                    t0 = blk * CBLK
                    xt = xc_pool.tile([128, KT, CBLK], F32R, tag="xc")
                    for kc in range(KT):
                        nc.sync.dma_start(
                            xt[:, kc, :],
                            x[t0 + 1:t0 + 1 + CBLK, kc * 128:(kc + 1) * 128]
                            .rearrange("t p -> p t").bitcast(F32R),
                        )
                    pt = blk_pool.tile([128, DT, CBLK], F32, tag="pt")
                    ct = blk_pool.tile([128, DT, CBLK], F32, tag="ct")
                    carry = xc_pool.tile([128, DT], F32, tag="carry")
                    gt = blk_pool.tile([128, DT, CBLK], F32, tag="gt")
                    lt = blk_pool.tile([128, DT, CBLK], F32R, tag="lt")
                    rt = blk_pool.tile([128, DT, CBLK], F32, tag="rt")
                    for a in range(DT):
                        ps = psc.tile([128, CBLK], F32, tag="ps")
                        for kc in range(KT):
                            nc.tensor.matmul(
                                ps[:], wog[:, kc, a, :], xt[:, kc, :],
                                start=(kc == 0), stop=(kc == KT - 1),
                            )
                        # P = (O + b_o) * glob
                        nc.vector.tensor_scalar(
                            pt[:, a, :], ps[:], bo_sb[:, a:a + 1],
                            glob[:, a:a + 1], OP.add, OP.mult,
                        )
                        init = (offset[:, a:a + 1] if c_prev is None
                                else c_prev[:, a:a + 1])
                        nc.vector.tensor_tensor_scan(
                            ct[:, a, :], pt[:, a, :], pt[:, a, :], init,
                            OP.add, OP.bypass,
                        )
                    # carry the last cumsum column via ACT so the next
                    # block's scan does not read a scan output directly
                    nc.scalar.copy(carry[:], ct[:, :, CBLK - 1:CBLK])
                    for a in range(DT):
                        ps = psc.tile([128, CBLK], F32, tag="ps")
                        for kc in range(KT):
                            nc.tensor.matmul(
                                ps[:], wog[:, kc, DT + a, :], xt[:, kc, :],
                                start=(kc == 0), stop=(kc == KT - 1),
                            )
                        sig = xc_pool.tile([128, CBLK], F32, tag="sig")
                        nc.scalar.activation(
                            sig[:], ps[:], AF.Sigmoid, bias=bg_sb[:, a:a + 1]
                        )
                        nc.vector.scalar_tensor_tensor(
                            gt[:, a, :], ps[:], bg_sb[:, a:a + 1], sig[:],
                            OP.add, OP.mult,
                        )
                        nc.vector.tensor_mul(lt[:, a, :], gt[:, a, :], ct[:, a, :])
                    for a in range(DT):
                        ps = psc.tile([128, CBLK], F32, tag="ps")
                        for kc in range(KT):
                            nc.tensor.matmul(
                                ps[:], wo2[:, kc, a, :], lt[:, kc, :],
                                start=(kc == 0), stop=(kc == KT - 1),
                            )
                        nc.vector.tensor_scalar_add(
                            rt[:, a, :], ps[:], bout_sb[:, a:a + 1]
                        )
                    for a in range(DT):
                        nc.sync.dma_start(
                            r_out[t0:t0 + CBLK, a * 128:(a + 1) * 128]
                            .rearrange("t p -> p t"),
                            rt[:, a, :],
                        )
                    c_prev = carry


_CACHE = {}


def _build(phases=5):
    if phases in _CACHE:
        return _CACHE[phases]
    nc = bacc.Bacc(None, target_bir_lowering=False, num_devices=N_CORES)
    prm = {
        "x": nc.declare_dram_parameter("x", [TH, DM], F32, isOutput=False),
        "wqt": nc.declare_dram_parameter("wqt", [DM, 3 * D], F32, isOutput=False),
        "wct": nc.declare_dram_parameter("wct", [3, D, D], F32, isOutput=False),
        "wat": nc.declare_dram_parameter("wat", [D, D], F32, isOutput=False),
        "wot": nc.declare_dram_parameter("wot", [D, D], F32, isOutput=False),
        "bq": nc.declare_dram_parameter("bq", [128, DT], F32, isOutput=False),
        "bo": nc.declare_dram_parameter("bo", [128, DT], F32, isOutput=False),
        "bg": nc.declare_dram_parameter("bg", [128, DT], F32, isOutput=False),
        "cb": nc.declare_dram_parameter("cb", [128, DT], F32, isOutput=False),
        "bout": nc.declare_dram_parameter("bout", [128, DT], F32, isOutput=False),
        "hf0": nc.declare_dram_parameter("hf0", [128, 1], F32, isOutput=False),
        "hf1": nc.declare_dram_parameter("hf1", [128, 1], F32, isOutput=False),
        "r": nc.declare_dram_parameter("r", [T, DM], F32, isOutput=True),
    }
    with tile.TileContext(nc, num_cores=N_CORES) as tc:
        _emit(tc, nc, prm, phases)
    nc.compile()
    _CACHE[phases] = nc
    return nc


def make_in_maps(x, W_qog, b_qog, conv_w, conv_b, w_a, W_out, b_out):
    f = np.float32
    x = np.asarray(x, f)
    wqt = np.ascontiguousarray(np.asarray(W_qog, f).T)          # [dm, 3d]
    wct = np.ascontiguousarray(np.asarray(conv_w, f).transpose(2, 1, 0))
    wat = np.ascontiguousarray(np.asarray(w_a, f).T)
    wot = np.ascontiguousarray(np.asarray(W_out, f).T)

    def col(v):  # [d] -> [128, DT] with d = a*128 + p
        return np.ascontiguousarray(np.asarray(v, f).reshape(DT, 128).T)

    b_qog = np.asarray(b_qog, f)
    bq, bo, bg = col(b_qog[:D]), col(b_qog[D:2 * D]), col(b_qog[2 * D:])
    cb, bout = col(conv_b), col(b_out)

    in_maps = []
    for c in range(N_CORES):
        b, h = c // 2, c % 2
        t0 = h * T
        xs = np.zeros((TH, DM), f)
        xs[1:T + 1] = x[b, t0:t0 + T]
        if t0 > 0:
            xs[0] = x[b, t0 - 1]
        if t0 + T < S:
            xs[T + 1] = x[b, t0 + T]
        in_maps.append({
            "x": xs, "wqt": wqt, "wct": wct, "wat": wat, "wot": wot,
            "bq": bq, "bo": bo, "bg": bg, "cb": cb, "bout": bout,
            "hf0": np.full((128, 1), 1.0 - h, f),
            "hf1": np.full((128, 1), float(h), f),
        })
    return in_maps


def kernel(x, W_qog, b_qog, conv_w, conv_b, w_a, W_out, b_out):
    nc = _build(5)
    in_maps = make_in_maps(x, W_qog, b_qog, conv_w, conv_b, w_a, W_out, b_out)
    res = run_bass_kernel_spmd(nc, in_maps, list(range(N_CORES)))
    out = np.empty((B, S, DM), np.float32)
    for c in range(N_CORES):
        b, h = c // 2, c % 2
        out[b, h * T:(h + 1) * T, :] = res.results[c]["r"]
    return out


# revision 15
# speedup vs baseline: 3.2919x; 3.2919x over previous
"""F2NetHead Trainium2 kernel (8 NeuronCores, Bass/Tile).

Reference computation (per batch b):
    qog = x @ W_qog.T + b_qog ; Q,O,G = split(qog)
    cq  = silu(conv1d(Q, conv_w, pad=1) + conv_b)          # mixes channels
    l   = (cq @ w_a.T) / sqrt(d)
    attn= softmax(l, axis=seq)
    glob= sum_seq(Q * attn)                                 # [1, d]
    P   = O * glob
    L   = silu(G) * cumsum(P, axis=seq)
    R   = L @ W_out.T + b_out

Sharding: 8 cores = 4 batches x 2 sequence halves. Each core computes
2048 tokens of one batch. The host supplies the x-rows with a 1-token
halo on each side (zero rows at the sequence edges) so the conv needs no
neighbor exchange. The only cross-core communication is a pairwise
AllReduce of 3 small [d] vectors per batch:
    E  = sum_seq exp(l)            (softmax denominator)
    N  = sum_seq Q * exp(l)        (softmax numerator of glob)
    sx = sum of this half's x rows (first half only)
The cumsum offset of the second half is glob * (W_O @ sx_h0 + T*b_O),
i.e. the first half's P-column-sums, computed without materializing O.

On-chip layout is feature-major ([d partitions, tokens free]) so every
sequence-axis op (softmax sums, global sum, cumsum) is a free-dim op.
All matmuls run in float32r (full PE rate, ~1.6e-4 rel err on hw).
"""

import numpy as np

import concourse.bacc as bacc
import concourse.mybir as mybir
import concourse.tile as tile
from concourse.bass_utils import run_bass_kernel_spmd

F32 = mybir.dt.float32
F32R = mybir.dt.float32r
AF = mybir.ActivationFunctionType
OP = mybir.AluOpType

B, S, D, DM = 4, 4096, 1024, 1024
N_CORES = 8
T = S // 2            # tokens per core
TH = T + 2            # with halo
DT = D // 128         # d tiles (8)
KT = DM // 128        # contraction tiles (8)
ABLK = 410            # phase A token block (5 blocks over TH=2050)
BBLK = 512            # phase B token block (4 blocks over T)
CBLK = 256            # phase C token block (8 blocks over T)
SCALE = 1.0 / float(np.sqrt(D))


def _emit(tc, nc, prm, phases=5):
    x, wqt, wct, wat, wot = prm["x"], prm["wqt"], prm["wct"], prm["wat"], prm["wot"]
    bq, bo, bg, cb, bout = prm["bq"], prm["bo"], prm["bg"], prm["cb"], prm["bout"]
    hf0, hf1, r_out = prm["hf0"], prm["hf1"], prm["r"]

    with (
        tc.tile_pool(name="cols", bufs=1) as cols,
        tc.tile_pool(name="dram", bufs=1, space="DRAM") as dram,
    ):
        # per-partition bias / flag columns ([128, DT] with d = a*128 + p)
        bq_sb = cols.tile([128, DT], F32)
        bo_sb = cols.tile([128, DT], F32)
        bg_sb = cols.tile([128, DT], F32)
        cb_sb = cols.tile([128, DT], F32)
        bout_sb = cols.tile([128, DT], F32)
        hf0_sb = cols.tile([128, 1], F32)
        hf1_sb = cols.tile([128, 1], F32)
        for t_, d_ in ((bq_sb, bq), (bo_sb, bo), (bg_sb, bg), (cb_sb, cb),
                       (bout_sb, bout), (hf0_sb, hf0), (hf1_sb, hf1)):
            nc.sync.dma_start(t_[:], d_[:])

        # accumulators that survive across phases
        sx_cols = cols.tile([128, KT, 5], F32)      # per-A-block x sums
        e_cols = cols.tile([128, DT * 4], F32)      # per-(a,B-block) exp sums
        n_cols = cols.tile([128, DT * 4], F32)      # per-(a,B-block) Q*exp sums
        stage = cols.tile([128, 3 * DT], F32)       # allreduce staging
        red = cols.tile([128, 3 * DT], F32)         # allreduce result
        glob = cols.tile([128, DT], F32)
        offset = cols.tile([128, DT], F32)

        # ---------------- phase A: Q^T over TH halo'd tokens ----------------
        with tc.tile_pool(name="qt", bufs=1) as qt_pool:
            qt = qt_pool.tile([128, DT, TH], F32R)
            with (
                tc.tile_pool(name="wq", bufs=1) as wq_pool,
                tc.tile_pool(name="xa", bufs=2) as xa_pool,
                tc.tile_pool(name="psa", bufs=8, space="PSUM") as psa,
            ):
                wq = wq_pool.tile([128, KT, DT, 128], F32R)
                for kc in range(KT):
                    nc.sync.dma_start(
                        wq[:, kc, :, :],
                        wqt[kc * 128:(kc + 1) * 128, 0:D]
                        .rearrange("p (a m) -> p a m", m=128).bitcast(F32R),
                    )
                for blk in range(5):
                    t0 = blk * ABLK
                    xt = xa_pool.tile([128, KT, ABLK], F32R, tag="xa")
                    for kc in range(KT):
                        nc.sync.dma_start(
                            xt[:, kc, :],
                            x[kc * 128:(kc + 1) * 128, t0:t0 + ABLK].bitcast(F32R),
                        )
                    # x column-sums over main (non-halo) tokens for cumsum offset
                    lo = 1 - t0 if t0 < 1 else 0
                    hi = ABLK - max(0, t0 + ABLK - (TH - 1))
                    nc.vector.tensor_reduce(
                        sx_cols[:, :, blk:blk + 1], xt[:, :, lo:hi],
                        axis=mybir.AxisListType.X, op=OP.add,
                    )
                    for a in range(DT):
                        ps = psa.tile([128, ABLK], F32, tag="ps")
                        for kc in range(KT):
                            nc.tensor.matmul(
                                ps[:], wq[:, kc, a, :], xt[:, kc, :],
                                start=(kc == 0), stop=(kc == KT - 1),
                            )
                        nc.vector.tensor_scalar_add(
                            qt[:, a, t0:t0 + ABLK], ps[:], bq_sb[:, a:a + 1]
                        )

            if phases == 1:
                for a in range(DT):
                    nc.sync.dma_start(
                        r_out[a * 128:(a + 1) * 128, 0:T],
                        qt[:, a, 1:T + 1].bitcast(F32),
                    )
                return

            # ------------- phase B1: cq^T = silu(conv(Q)) -------------
            with tc.tile_pool(name="cq", bufs=1) as cq_pool:
                cq = cq_pool.tile([128, DT, T], F32R)
                with (
                    tc.tile_pool(name="wc", bufs=2) as wc_pool,
                    tc.tile_pool(name="psb", bufs=8, space="PSUM") as psb,
                ):
                    for a in range(DT):
                        wc = wc_pool.tile([128, 3, KT, 128], F32R, tag="wc")
                        for k3 in range(3):
                            nc.sync.dma_start(
                                wc[:, k3, :, :],
                                wct[k3, :, a * 128:(a + 1) * 128]
                                .rearrange("(kc p) m -> p kc m", p=128)
                                .bitcast(F32R),
                            )
                        for blk in range(T // BBLK):
                            t0 = blk * BBLK
                            ps = psb.tile([128, BBLK], F32, tag="ps")
                            first = True
                            for k3 in range(3):
                                for kc in range(KT):
                                    nc.tensor.matmul(
                                        ps[:], wc[:, k3, kc, :],
                                        qt[:, kc, t0 + k3:t0 + k3 + BBLK],
                                        start=first,
                                        stop=(k3 == 2 and kc == KT - 1),
                                    )
                                    first = False
                            sig = wc_pool.tile([128, BBLK], F32, tag="sig")
                            nc.scalar.activation(
                                sig[:], ps[:], AF.Sigmoid, bias=cb_sb[:, a:a + 1]
                            )
                            nc.vector.scalar_tensor_tensor(
                                cq[:, a, t0:t0 + BBLK], ps[:], cb_sb[:, a:a + 1],
                                sig[:], OP.add, OP.mult,
                            )

                if phases == 2:
                    for a in range(DT):
                        nc.sync.dma_start(
                            r_out[a * 128:(a + 1) * 128, 0:T],
                            cq[:, a, :].bitcast(F32),
                        )
                    return

                # ------- phase B2: E/N partial sums from exp(logits) -------
                with (
                    tc.tile_pool(name="wa", bufs=1) as wa_pool,
                    tc.tile_pool(name="ex", bufs=4) as ex_pool,
                    tc.tile_pool(name="psl", bufs=8, space="PSUM") as psl,
                ):
                    wa = wa_pool.tile([128, KT, DT, 128], F32R)
                    for kc in range(KT):
                        nc.sync.dma_start(
                            wa[:, kc, :, :],
                            wat[kc * 128:(kc + 1) * 128, :]
                            .rearrange("p (a m) -> p a m", m=128).bitcast(F32R),
                        )
                    for blk in range(T // BBLK):
                        t0 = blk * BBLK
                        for a in range(DT):
                            ps = psl.tile([128, BBLK], F32, tag="ps")
                            for kc in range(KT):
                                nc.tensor.matmul(
                                    ps[:], wa[:, kc, a, :], cq[:, kc, t0:t0 + BBLK],
                                    start=(kc == 0), stop=(kc == KT - 1),
                                )
                            expl = ex_pool.tile([128, BBLK], F32, tag="expl")
                            idx = a * 4 + blk
                            nc.scalar.activation(
                                expl[:], ps[:], AF.Exp, scale=SCALE,
                                accum_out=e_cols[:, idx:idx + 1],
                            )
                            prod = ex_pool.tile([128, BBLK], F32, tag="prod")
                            nc.vector.scalar_tensor_tensor(
                                prod[:], expl[:], 0.0,
                                qt[:, a, t0 + 1:t0 + 1 + BBLK].bitcast(F32),
                                OP.add, OP.mult,
                                accum_out=n_cols[:, idx:idx + 1],
                            )

        if phases == 3:
            nc.sync.dma_start(r_out[0:128, 0:32].rearrange("p t -> p t"), e_cols[:])
            nc.sync.dma_start(r_out[128:256, 0:32], n_cols[:])
            return

        # ---------------- allreduce E, N, sx over the seq pair ----------------
        nc.vector.tensor_reduce(
            stage[:, 0:DT], e_cols[:].rearrange("p (a b) -> p a b", b=4),
            axis=mybir.AxisListType.X, op=OP.add,
        )
        nc.vector.tensor_reduce(
            stage[:, DT:2 * DT], n_cols[:].rearrange("p (a b) -> p a b", b=4),
            axis=mybir.AxisListType.X, op=OP.add,
        )
        # x sums (main tokens) masked to the first half: slot = sx * (1-h)
        nc.vector.tensor_reduce(
            stage[:, 2 * DT:3 * DT], sx_cols[:],
            axis=mybir.AxisListType.X, op=OP.add,
        )
        nc.vector.tensor_scalar_mul(
            stage[:, 2 * DT:3 * DT], stage[:, 2 * DT:3 * DT], hf0_sb[:, 0:1]
        )
        cc_in = dram.tile([128, 3 * DT], F32)
        cc_out = dram.tile([128, 3 * DT], F32)
        nc.sync.dma_start(cc_in[:], stage[:])
        nc.gpsimd.collective_compute(
            "AllReduce", OP.add,
            replica_groups=[[0, 1], [2, 3], [4, 5], [6, 7]],
            ins=[cc_in.opt()], outs=[cc_out.opt()],
        )
        nc.sync.dma_start(red[:], cc_out[:])

        # glob = N / E
        recip = cols.tile([128, DT], F32)
        nc.vector.reciprocal(recip[:], red[:, 0:DT])
        nc.vector.tensor_mul(glob[:], red[:, DT:2 * DT], recip[:])

        # ---------------- phase C: O,G -> P -> cumsum -> L -> R ----------------
        with (
            tc.tile_pool(name="wog", bufs=1) as wog_pool,
            tc.tile_pool(name="wo2", bufs=1) as wo2_pool,
        ):
            wog = wog_pool.tile([128, KT, 2 * DT, 128], F32R)
            for kc in range(KT):
                nc.sync.dma_start(
                    wog[:, kc, :, :],
                    wqt[kc * 128:(kc + 1) * 128, D:3 * D]
                    .rearrange("p (a m) -> p a m", m=128).bitcast(F32R),
                )
            wo2 = wo2_pool.tile([128, KT, DT, 128], F32R)
            for kc in range(KT):
                nc.sync.dma_start(
                    wo2[:, kc, :, :],
                    wot[kc * 128:(kc + 1) * 128, :]
                    .rearrange("p (a m) -> p a m", m=128).bitcast(F32R),
                )

            # cumsum offset for the second half: glob * (W_O @ sx_h0 + T*b_O)
            # (plain fp32 matmul: fp32r rejects a size-1 moving operand)
            bo_t = cols.tile([128, DT], F32)
            nc.vector.tensor_scalar_mul(bo_t[:], bo_sb[:], float(T))
            offv = cols.tile([128, DT], F32)
            with tc.tile_pool(name="psm", bufs=2, space="PSUM") as psm:
                for a in range(DT):
                    ps = psm.tile([128, 1], F32, tag="ps")
                    for kc in range(KT):
                        nc.tensor.matmul(
                            ps[:], wog[:, kc, a, :].bitcast(F32),
                            red[:, 2 * DT + kc:2 * DT + kc + 1],
                            start=(kc == 0), stop=(kc == KT - 1),
                        )
                    nc.vector.tensor_scalar_add(
                        offv[:, a:a + 1], ps[:], bo_t[:, a:a + 1]
                    )
            nc.vector.tensor_mul(offset[:], offv[:], glob[:])
            nc.vector.tensor_scalar_mul(offset[:], offset[:], hf1_sb[:, 0:1])

            if phases == 4:
                nc.sync.dma_start(r_out[0:128, 0:DT], offset[:])
                nc.sync.dma_start(r_out[128:256, 0:DT], glob[:])
                return

            with (
                tc.tile_pool(name="xc", bufs=2) as xc_pool,
                tc.tile_pool(name="blkb", bufs=2) as blk_pool,
                tc.tile_pool(name="psc", bufs=8, space="PSUM") as psc,
            ):
                c_prev = None
                nblk = T // CBLK if phases >= 5 else 0
                if phases >= 50:
                    nblk = phases - 50
                for blk in range(nblk):
                    t0 = blk * CBLK
                    xt = xc_pool.tile([128, KT, CBLK], F32R, tag="xc")
                    for kc in range(KT):
                        nc.sync.dma_start(
                            xt[:, kc, :],
                            x[kc * 128:(kc + 1) * 128, t0 + 1:t0 + 1 + CBLK]
                            .bitcast(F32R),
                        )
                    pt = blk_pool.tile([128, DT, CBLK], F32, tag="pt")
                    ct = blk_pool.tile([128, DT, CBLK], F32, tag="ct")
                    carry = xc_pool.tile([128, DT], F32, tag="carry")
                    gt = blk_pool.tile([128, DT, CBLK], F32, tag="gt")
                    lt = blk_pool.tile([128, DT, CBLK], F32R, tag="lt")
                    rt = blk_pool.tile([128, DT, CBLK], F32, tag="rt")
                    for a in range(DT):
                        ps = psc.tile([128, CBLK], F32, tag="ps")
                        for kc in range(KT):
                            nc.tensor.matmul(
                                ps[:], wog[:, kc, a, :], xt[:, kc, :],
                                start=(kc == 0), stop=(kc == KT - 1),
                            )
                        # P = (O + b_o) * glob
                        nc.vector.tensor_scalar(
                            pt[:, a, :], ps[:], bo_sb[:, a:a + 1],
                            glob[:, a:a + 1], OP.add, OP.mult,
                        )
                        init = (offset[:, a:a + 1] if c_prev is None
                                else c_prev[:, a:a + 1])
                        nc.vector.tensor_tensor_scan(
                            ct[:, a, :], pt[:, a, :], pt[:, a, :], init,
                            OP.add, OP.bypass,
                        )
                    # carry the last cumsum column via ACT so the next
                    # block's scan does not read a scan output directly
                    nc.scalar.copy(carry[:], ct[:, :, CBLK - 1:CBLK])
                    for a in range(DT):
                        ps = psc.tile([128, CBLK], F32, tag="ps")
                        for kc in range(KT):
                            nc.tensor.matmul(
                                ps[:], wog[:, kc, DT + a, :], xt[:, kc, :],
                                start=(kc == 0), stop=(kc == KT - 1),
                            )
                        sig = xc_pool.tile([128, CBLK], F32, tag="sig")
                        nc.scalar.activation(
                            sig[:], ps[:], AF.Sigmoid, bias=bg_sb[:, a:a + 1]
                        )
                        nc.vector.scalar_tensor_tensor(
                            gt[:, a, :], ps[:], bg_sb[:, a:a + 1], sig[:],
                            OP.add, OP.mult,
                        )
                        nc.vector.tensor_mul(lt[:, a, :], gt[:, a, :], ct[:, a, :])
                    for a in range(DT):
                        ps = psc.tile([128, CBLK], F32, tag="ps")
                        for kc in range(KT):
                            nc.tensor.matmul(
                                ps[:], wo2[:, kc, a, :], lt[:, kc, :],
                                start=(kc == 0), stop=(kc == KT - 1),
                            )
                        nc.vector.tensor_scalar_add(
                            rt[:, a, :], ps[:], bout_sb[:, a:a + 1]
                        )
                    for a in range(DT):
                        nc.sync.dma_start(
                            r_out[a * 128:(a + 1) * 128, t0:t0 + CBLK],
                            rt[:, a, :],
                        )
                    c_prev = carry


_CACHE = {}


def _build(phases=5):
    if phases in _CACHE:
        return _CACHE[phases]
    nc = bacc.Bacc(None, target_bir_lowering=False, num_devices=N_CORES)
    prm = {
        "x": nc.declare_dram_parameter("x", [DM, TH], F32, isOutput=False),
        "wqt": nc.declare_dram_parameter("wqt", [DM, 3 * D], F32, isOutput=False),
        "wct": nc.declare_dram_parameter("wct", [3, D, D], F32, isOutput=False),
        "wat": nc.declare_dram_parameter("wat", [D, D], F32, isOutput=False),
        "wot": nc.declare_dram_parameter("wot", [D, D], F32, isOutput=False),
        "bq": nc.declare_dram_parameter("bq", [128, DT], F32, isOutput=False),
        "bo": nc.declare_dram_parameter("bo", [128, DT], F32, isOutput=False),
        "bg": nc.declare_dram_parameter("bg", [128, DT], F32, isOutput=False),
        "cb": nc.declare_dram_parameter("cb", [128, DT], F32, isOutput=False),
        "bout": nc.declare_dram_parameter("bout", [128, DT], F32, isOutput=False),
        "hf0": nc.declare_dram_parameter("hf0", [128, 1], F32, isOutput=False),
        "hf1": nc.declare_dram_parameter("hf1", [128, 1], F32, isOutput=False),
        "r": nc.declare_dram_parameter("r", [DM, T], F32, isOutput=True),
    }
    with tile.TileContext(nc, num_cores=N_CORES) as tc:
        _emit(tc, nc, prm, phases)
    nc.compile()
    _CACHE[phases] = nc
    return nc


def make_in_maps(x, W_qog, b_qog, conv_w, conv_b, w_a, W_out, b_out):
    f = np.float32
    x = np.asarray(x, f)
    wqt = np.ascontiguousarray(np.asarray(W_qog, f).T)          # [dm, 3d]
    wct = np.ascontiguousarray(np.asarray(conv_w, f).transpose(2, 1, 0))
    wat = np.ascontiguousarray(np.asarray(w_a, f).T)
    wot = np.ascontiguousarray(np.asarray(W_out, f).T)

    def col(v):  # [d] -> [128, DT] with d = a*128 + p
        return np.ascontiguousarray(np.asarray(v, f).reshape(DT, 128).T)

    b_qog = np.asarray(b_qog, f)
    bq, bo, bg = col(b_qog[:D]), col(b_qog[D:2 * D]), col(b_qog[2 * D:])
    cb, bout = col(conv_b), col(b_out)

    in_maps = []
    for c in range(N_CORES):
        b, h = c // 2, c % 2
        t0 = h * T
        xs = np.zeros((TH, DM), f)
        xs[1:T + 1] = x[b, t0:t0 + T]
        if t0 > 0:
            xs[0] = x[b, t0 - 1]
        if t0 + T < S:
            xs[T + 1] = x[b, t0 + T]
        xs = np.ascontiguousarray(xs.T)            # [DM, TH] feature-major
        in_maps.append({
            "x": xs, "wqt": wqt, "wct": wct, "wat": wat, "wot": wot,
            "bq": bq, "bo": bo, "bg": bg, "cb": cb, "bout": bout,
            "hf0": np.full((128, 1), 1.0 - h, f),
            "hf1": np.full((128, 1), float(h), f),
        })
    return in_maps


def kernel(x, W_qog, b_qog, conv_w, conv_b, w_a, W_out, b_out):
    nc = _build(5)
    in_maps = make_in_maps(x, W_qog, b_qog, conv_w, conv_b, w_a, W_out, b_out)
    res = run_bass_kernel_spmd(nc, in_maps, list(range(N_CORES)))
    out = np.empty((B, S, DM), np.float32)
    for c in range(N_CORES):
        b, h = c // 2, c % 2
        out[b, h * T:(h + 1) * T, :] = res.results[c]["r"].T
    return out
